# revision 11
# baseline (speedup 1.0000x reference)
"""Trainium2 Bass kernel for nn_Adaptive_FP (retrieval KNN), 8 NeuronCores.

Sharding: data-parallel over B (8 batches -> 8 cores). Per core:
  - feats = features.T              via PE transposes
  - x = feats @ W1.T + b1           via PE matmul (ones-row bias trick)
  - knn: EXACT fp32 d2 for all 2048 candidates per query tile:
      ACT computes (q_c - p_c)^2 per coordinate (same fp32 rounding as the
      reference), DMA CCE-add accumulates the three squares, DVE negates
      (2x mode) and runs the flat top-16 via max8/max_index/match_replace
      (positions are global indices, already rank-ordered), gpsimd ap_gather
      fetches winner xyz rows (16-partition-group shared lists), and a
      mask-multiply + reduce-add extracts each query's own rows.
knn matches stable argsort of the reference's fp32 d2 bit-exactly (up to
fp32-tie degeneracies).
"""
import sys
import numpy as np

sys.path.insert(0, "/opt/trn_rl_repo")

def _install_ntff_hook_shim():
    """Provide antenv.axon_hooks (absent from this image's antenv stub) so
    run_bass_kernel_spmd(trace=True) can fetch the NTFF profiling hook."""
    import types
    if "antenv.axon_hooks" in sys.modules:
        return
    mod = types.ModuleType("antenv.axon_hooks")
    state = {"hook": None, "tried": False}

    def set_axon_ntff_profile_hook(hook):
        state["hook"] = hook

    def get_axon_ntff_profile_hook():
        if state["hook"] is None and not state["tried"]:
            state["tried"] = True
            try:
                sys.path.insert(0, "/root/.axon_site/trn_agent_boot")
                from trn_boot import _ntff_profile_via_ctypes
                state["hook"] = _ntff_profile_via_ctypes(
                    "/opt/axon/libaxon_pjrt.so")
            except Exception:
                state["hook"] = None
        return state["hook"]

    mod.set_axon_ntff_profile_hook = set_axon_ntff_profile_hook
    mod.get_axon_ntff_profile_hook = get_axon_ntff_profile_hook
    sys.modules["antenv.axon_hooks"] = mod
    try:
        import antenv
        antenv.axon_hooks = mod
    except Exception:
        pass


_install_ntff_hook_shim()

import concourse.bass as bass
import concourse.bacc as bacc
import concourse.mybir as mybir
from concourse.tile import TileContext
from concourse.bass_utils import run_bass_kernel_spmd

F32 = mybir.dt.float32
U16 = mybir.dt.uint16
I16 = mybir.dt.int16
ALU = mybir.AluOpType
AXX = mybir.AxisListType.X
SQUARE = mybir.ActivationFunctionType.Square
CPY = mybir.ActivationFunctionType.Copy

B, N, M, CP, DM, KNN = 8, 2048, 8192, 64, 64, 16
NT = M // 128            # 64 query tiles per core
NEG = -3.0e38            # finite -inf sentinel for match_replace


def build_nc():
    nc = bacc.Bacc()

    # ---- external inputs ----
    xyzb_d = nc.declare_dram_parameter("xyzb", [128, N, 3], F32, isOutput=False)
    xyzp_d = nc.declare_dram_parameter("xyzp", [128, 3, N], F32, isOutput=False)
    qbuf_d = nc.declare_dram_parameter("qbuf", [128, NT, 3], F32, isOutput=False)
    mask_d = nc.declare_dram_parameter("maskx", [128, KNN * 16 * 3], F32,
                                       isOutput=False)
    feat_d = nc.declare_dram_parameter("feat65", [CP + 1, N], F32, isOutput=False)
    w1_d = nc.declare_dram_parameter("w1tb1", [CP + 1, DM], F32, isOutput=False)
    id_d = nc.declare_dram_parameter("ident", [64, 64], F32, isOutput=False)

    # ---- external outputs ----
    knn_o = nc.declare_dram_parameter("knn_out", [128, NT, KNN, 3], F32,
                                      isOutput=True)
    feats_o = nc.declare_dram_parameter("feats_out", [128, N // 128, DM], F32,
                                        isOutput=True)
    x_o = nc.declare_dram_parameter("x_out", [128, N // 128, DM], F32,
                                    isOutput=True)

    with TileContext(nc) as tc:
        with tc.tile_pool(name="cn", bufs=1) as cn, \
             tc.tile_pool(name="io", bufs=1) as io, \
             tc.tile_pool(name="work", bufs=2) as wk, \
             tc.tile_pool(name="wsm", bufs=3) as wsm, \
             tc.tile_pool(name="psS", bufs=2, space="PSUM") as psS:

            # ---------------- load inputs ----------------
            xyzb = cn.tile([128, N, 3], F32)
            xyzp = cn.tile([128, 3, N], F32)
            qbuf = cn.tile([128, NT, 3], F32)
            maskx = cn.tile([128, KNN * 16 * 3], F32)
            feat_s = cn.tile([CP + 1, N], F32)
            w1_s = cn.tile([CP + 1, DM], F32)
            id_s = cn.tile([64, 64], F32)
            for dst, src in [(xyzb, xyzb_d), (xyzp, xyzp_d),
                             (qbuf, qbuf_d), (maskx, mask_d),
                             (feat_s, feat_d), (w1_s, w1_d), (id_s, id_d)]:
                nc.sync.dma_start(out=dst[:], in_=src[:])

            # ---------------- feats / x (PE) ----------------
            featsb = io.tile([128, N // 128, DM], F32)
            xb = io.tile([128, N // 128, DM], F32)
            for kk in range(N // 128):
                psf = psS.tile([128, DM], F32, tag="psf")
                nc.tensor.transpose(psf[:], feat_s[0:CP, kk * 128:(kk + 1) * 128],
                                    id_s[:])
                nc.scalar.activation(out=featsb[:, kk], in_=psf[:], func=CPY)
                psx = psS.tile([128, DM], F32, tag="psx")
                nc.tensor.matmul(psx[:], feat_s[:, kk * 128:(kk + 1) * 128],
                                 w1_s[:], start=True, stop=True)
                nc.scalar.activation(out=xb[:, kk], in_=psx[:], func=CPY)
            nc.sync.dma_start(out=feats_o[:], in_=featsb[:])
            nc.sync.dma_start(out=x_o[:], in_=xb[:])

            # ---------------- main loop ----------------
            knn_buf = io.tile([128, NT, KNN, 3], F32)
            for t in range(NT):
                # exact squares per coordinate; acc accumulates via DMA CCE add
                acc = wk.tile([128, N], F32, tag="acc", bufs=3)
                sq1 = wk.tile([128, N], F32, tag="sq1", bufs=3)
                sq2 = wk.tile([128, N], F32, tag="sq2", bufs=3)
                for c, out_t in ((0, acc), (1, sq1), (2, sq2)):
                    nc.scalar.activation(
                        out=out_t[:], in_=xyzp[:, c], func=SQUARE,
                        bias=qbuf[:, t, c:c + 1], scale=-1.0)
                # acc = (sq0 + sq1) via DMA CCE add (reference rounding order)
                nc.gpsimd.dma_start(out=acc[:], in_=sq1[:], accum_op=ALU.add)
                # negd2 = (-acc) - sq2 == -((sq0+sq1)+sq2), exact
                negd2 = wk.tile([128, N], F32, tag="negd2")
                nc.vector.scalar_tensor_tensor(
                    out=negd2[:], in0=acc[:], scalar=-1.0, in1=sq2[:],
                    op0=ALU.mult, op1=ALU.subtract)

                # flat exact top-16: positions are global ids, rank-ordered
                v8 = wsm.tile([128, KNN], F32, tag="v8")
                pos16 = wsm.tile([128, KNN], U16, tag="pos16")
                nc.vector.max(v8[:, 0:8], negd2[:])
                nc.vector.max_index(pos16[:, 0:8], v8[:, 0:8], negd2[:])
                nc.vector.match_replace(negd2[:], v8[:, 0:8], negd2[:], NEG)
                nc.vector.max(v8[:, 8:16], negd2[:])
                nc.vector.max_index(pos16[:, 8:16], v8[:, 8:16], negd2[:])

                # gather winner coords (16-partition-group shared lists)
                gath = wsm.tile([128, KNN * 16, 3], F32, tag="gath")
                nc.gpsimd.ap_gather(out_ap=gath[:], in_ap=xyzb[:],
                                    idxs_ap=pos16.bitcast(I16)[:],
                                    channels=128, num_elems=N, d=3,
                                    num_idxs=KNN * 16)
                # extract own query's rows: multiply by one-hot mask, reduce q'
                tmp = wsm.tile([128, KNN * 16, 3], F32, tag="tmp")
                nc.gpsimd.tensor_tensor(
                    out=tmp.rearrange("p i c -> p (i c)"),
                    in0=gath.rearrange("p i c -> p (i c)"),
                    in1=maskx[:], op=ALU.mult)
                nc.vector.tensor_reduce(
                    out=knn_buf[:, t],
                    in_=tmp.rearrange("p (j q) c -> p j c q", q=16),
                    op=ALU.add, axis=AXX)

            nc.sync.dma_start(out=knn_o[:], in_=knn_buf[:])

    nc.compile()
    return nc


_NC_CACHE = {}


def _get_nc():
    if "nc" not in _NC_CACHE:
        _NC_CACHE["nc"] = build_nc()
    return _NC_CACHE["nc"]


def _prep_in_maps(xyz, xyz_fp, features, W1, b1):
    xyz = np.asarray(xyz, np.float32)
    xyz_fp = np.asarray(xyz_fp, np.float32)
    features = np.asarray(features, np.float32)
    W1 = np.asarray(W1, np.float32)
    b1 = np.asarray(b1, np.float32)

    w1tb1 = np.concatenate([W1.T, b1[None, :]], 0).astype(np.float32)
    ident = np.eye(64, dtype=np.float32)
    maskx = np.zeros((128, KNN, 16, 3), np.float32)
    for p in range(128):
        maskx[p, :, p % 16, :] = 1.0
    maskx = maskx.reshape(128, KNN * 16 * 3)

    in_maps = []
    for b in range(B):
        xb = np.ascontiguousarray(xyz[b])
        fb = np.concatenate([features[b], np.ones((1, N), np.float32)], 0)
        in_maps.append({
            "xyzb": np.ascontiguousarray(np.tile(xb[None], (128, 1, 1))),
            "xyzp": np.ascontiguousarray(np.tile(xb.T[None], (128, 1, 1))),
            "qbuf": np.ascontiguousarray(
                xyz_fp[b].reshape(NT, 128, 3).transpose(1, 0, 2)),
            "maskx": maskx,
            "feat65": np.ascontiguousarray(fb),
            "w1tb1": w1tb1,
            "ident": ident,
        })
    return in_maps


def run(xyz, xyz_fp, features, W1, b1, trace=False):
    nc = _get_nc()
    in_maps = _prep_in_maps(xyz, xyz_fp, features, W1, b1)
    res = run_bass_kernel_spmd(nc, in_maps, core_ids=list(range(B)), trace=trace)
    feats = np.empty((B, N, DM), np.float32)
    knn = np.empty((B, M, KNN, 3), np.float32)
    x = np.empty((B, N, DM), np.float32)
    for b in range(B):
        r = res.results[b]
        feats[b] = r["feats_out"].transpose(1, 0, 2).reshape(N, DM)
        x[b] = r["x_out"].transpose(1, 0, 2).reshape(N, DM)
        knn[b] = r["knn_out"].transpose(1, 0, 2, 3).reshape(M, KNN, 3)
    return (feats, knn, x), res


def kernel(xyz, xyz_fp, features, features_fp=None, W1=None, b1=None, k=16,
           **_ignored):
    assert int(k) == KNN
    (feats, knn, x), _ = run(xyz, xyz_fp, features, W1, b1, trace=False)
    return feats, knn, x


# revision 12
# speedup vs baseline: 1.1248x; 1.1248x over previous
"""Trainium2 Bass kernel for nn_Adaptive_FP (retrieval KNN), 8 NeuronCores.

Sharding: data-parallel over B (8 batches -> 8 cores). Per core:
  - feats = features.T              via PE transposes
  - x = feats @ W1.T + b1           via PE matmul (ones-row bias trick)
  - knn: EXACT fp32 d2 for all 2048 candidates per query tile:
      ACT computes (q_c - p_c)^2 per coordinate (same fp32 rounding as the
      reference), DMA CCE-add accumulates the three squares, DVE negates
      (2x mode) and runs the flat top-16 via max8/max_index/match_replace
      (positions are global indices, already rank-ordered), gpsimd ap_gather
      fetches winner xyz rows (16-partition-group shared lists), and a
      mask-multiply + reduce-add extracts each query's own rows.
knn matches stable argsort of the reference's fp32 d2 bit-exactly (up to
fp32-tie degeneracies).
"""
import sys
import numpy as np

sys.path.insert(0, "/opt/trn_rl_repo")

def _install_ntff_hook_shim():
    """Provide antenv.axon_hooks (absent from this image's antenv stub) so
    run_bass_kernel_spmd(trace=True) can fetch the NTFF profiling hook."""
    import types
    if "antenv.axon_hooks" in sys.modules:
        return
    mod = types.ModuleType("antenv.axon_hooks")
    state = {"hook": None, "tried": False}

    def set_axon_ntff_profile_hook(hook):
        state["hook"] = hook

    def get_axon_ntff_profile_hook():
        if state["hook"] is None and not state["tried"]:
            state["tried"] = True
            try:
                sys.path.insert(0, "/root/.axon_site/trn_agent_boot")
                from trn_boot import _ntff_profile_via_ctypes
                state["hook"] = _ntff_profile_via_ctypes(
                    "/opt/axon/libaxon_pjrt.so")
            except Exception:
                state["hook"] = None
        return state["hook"]

    mod.set_axon_ntff_profile_hook = set_axon_ntff_profile_hook
    mod.get_axon_ntff_profile_hook = get_axon_ntff_profile_hook
    sys.modules["antenv.axon_hooks"] = mod
    try:
        import antenv
        antenv.axon_hooks = mod
    except Exception:
        pass


_install_ntff_hook_shim()

import concourse.bass as bass
import concourse.bacc as bacc
import concourse.mybir as mybir
from concourse.tile import TileContext
from concourse.bass_utils import run_bass_kernel_spmd

F32 = mybir.dt.float32
U16 = mybir.dt.uint16
I16 = mybir.dt.int16
ALU = mybir.AluOpType
AXX = mybir.AxisListType.X
SQUARE = mybir.ActivationFunctionType.Square
CPY = mybir.ActivationFunctionType.Copy

B, N, M, CP, DM, KNN = 8, 2048, 8192, 64, 64, 16
NT = M // 128            # 64 query tiles per core
NEG = -3.0e38            # finite -inf sentinel for match_replace


def build_nc():
    nc = bacc.Bacc()

    # ---- external inputs ----
    xyzb_d = nc.declare_dram_parameter("xyzb", [128, N, 3], F32, isOutput=False)
    xyzp_d = nc.declare_dram_parameter("xyzp", [128, 3, N], F32, isOutput=False)
    qbuf_d = nc.declare_dram_parameter("qbuf", [128, NT, 3], F32, isOutput=False)
    mask_d = nc.declare_dram_parameter("maskx", [128, KNN * 16 * 3], F32,
                                       isOutput=False)
    feat_d = nc.declare_dram_parameter("feat65", [CP + 1, N], F32, isOutput=False)
    w1_d = nc.declare_dram_parameter("w1tb1", [CP + 1, DM], F32, isOutput=False)
    id_d = nc.declare_dram_parameter("ident", [64, 64], F32, isOutput=False)

    # ---- external outputs ----
    knn_o = nc.declare_dram_parameter("knn_out", [128, NT, KNN, 3], F32,
                                      isOutput=True)
    feats_o = nc.declare_dram_parameter("feats_out", [128, N // 128, DM], F32,
                                        isOutput=True)
    x_o = nc.declare_dram_parameter("x_out", [128, N // 128, DM], F32,
                                    isOutput=True)

    with TileContext(nc) as tc:
        with tc.tile_pool(name="cn", bufs=1) as cn, \
             tc.tile_pool(name="io", bufs=1) as io, \
             tc.tile_pool(name="work", bufs=2) as wk, \
             tc.tile_pool(name="wsm", bufs=2) as wsm, \
             tc.tile_pool(name="psS", bufs=2, space="PSUM") as psS:

            # ---------------- load inputs ----------------
            xyzb = cn.tile([128, N, 3], F32)
            xyzp = cn.tile([128, 3, N], F32)
            qbuf = cn.tile([128, NT, 3], F32)
            maskx = cn.tile([128, KNN * 16 * 3], F32)
            feat_s = cn.tile([CP + 1, N], F32)
            w1_s = cn.tile([CP + 1, DM], F32)
            id_s = cn.tile([64, 64], F32)
            for dst, src in [(xyzb, xyzb_d), (xyzp, xyzp_d),
                             (qbuf, qbuf_d), (maskx, mask_d),
                             (feat_s, feat_d), (w1_s, w1_d), (id_s, id_d)]:
                nc.sync.dma_start(out=dst[:], in_=src[:])

            # ---------------- feats / x (PE) ----------------
            featsb = io.tile([128, N // 128, DM], F32)
            xb = io.tile([128, N // 128, DM], F32)
            for kk in range(N // 128):
                psf = psS.tile([128, DM], F32, tag="psf")
                nc.tensor.transpose(psf[:], feat_s[0:CP, kk * 128:(kk + 1) * 128],
                                    id_s[:])
                nc.scalar.activation(out=featsb[:, kk], in_=psf[:], func=CPY)
                psx = psS.tile([128, DM], F32, tag="psx")
                nc.tensor.matmul(psx[:], feat_s[:, kk * 128:(kk + 1) * 128],
                                 w1_s[:], start=True, stop=True)
                nc.scalar.activation(out=xb[:, kk], in_=psx[:], func=CPY)
            nc.sync.dma_start(out=feats_o[:], in_=featsb[:])
            nc.sync.dma_start(out=x_o[:], in_=xb[:])

            # ---------------- main loop ----------------
            knn_buf = io.tile([128, NT, KNN, 3], F32)
            for t in range(NT):
                # exact squares per coordinate; acc accumulates via DMA CCE add
                acc = wk.tile([128, N], F32, tag="acc", bufs=3)
                sq1 = wk.tile([128, N], F32, tag="sq1", bufs=3)
                sq2 = wk.tile([128, N], F32, tag="sq2", bufs=3)
                for c, out_t in ((0, acc), (1, sq1), (2, sq2)):
                    nc.scalar.activation(
                        out=out_t[:], in_=xyzp[:, c], func=SQUARE,
                        bias=qbuf[:, t, c:c + 1], scale=-1.0)
                # acc = (sq0 + sq1) via DMA CCE add (reference rounding order)
                nc.gpsimd.dma_start(out=acc[:], in_=sq1[:], accum_op=ALU.add)
                # negd2 = (-acc) - sq2 == -((sq0+sq1)+sq2), exact
                negd2 = wk.tile([128, N], F32, tag="negd2", bufs=3)
                nc.vector.scalar_tensor_tensor(
                    out=negd2[:], in0=acc[:], scalar=-1.0, in1=sq2[:],
                    op0=ALU.mult, op1=ALU.subtract)

                # flat exact top-16: positions are global ids, rank-ordered
                v8 = wsm.tile([128, KNN], F32, tag="v8")
                pos16 = wsm.tile([128, KNN], U16, tag="pos16")
                nc.vector.max(v8[:, 0:8], negd2[:])
                nc.vector.max_index(pos16[:, 0:8], v8[:, 0:8], negd2[:])
                nc.vector.match_replace(negd2[:], v8[:, 0:8], negd2[:], NEG)
                nc.vector.max(v8[:, 8:16], negd2[:])
                nc.vector.max_index(pos16[:, 8:16], v8[:, 8:16], negd2[:])

                # gather winner coords (16-partition-group shared lists)
                gath = wsm.tile([128, KNN * 16, 3], F32, tag="gath")
                nc.gpsimd.ap_gather(out_ap=gath[:], in_ap=xyzb[:],
                                    idxs_ap=pos16.bitcast(I16)[:],
                                    channels=128, num_elems=N, d=3,
                                    num_idxs=KNN * 16)
                # extract own query's rows: multiply by one-hot mask, reduce q'
                tmp = wsm.tile([128, KNN * 16, 3], F32, tag="tmp")
                nc.vector.scalar_tensor_tensor(
                    out=tmp.rearrange("p i c -> p (i c)"),
                    in0=gath.rearrange("p i c -> p (i c)"), scalar=0.0,
                    in1=maskx[:], op0=ALU.add, op1=ALU.mult)
                nc.vector.tensor_reduce(
                    out=knn_buf[:, t],
                    in_=tmp.rearrange("p (j q) c -> p j c q", q=16),
                    op=ALU.add, axis=AXX)

            nc.sync.dma_start(out=knn_o[:], in_=knn_buf[:])

    nc.compile()
    return nc


_NC_CACHE = {}


def _get_nc():
    if "nc" not in _NC_CACHE:
        _NC_CACHE["nc"] = build_nc()
    return _NC_CACHE["nc"]


def _prep_in_maps(xyz, xyz_fp, features, W1, b1):
    xyz = np.asarray(xyz, np.float32)
    xyz_fp = np.asarray(xyz_fp, np.float32)
    features = np.asarray(features, np.float32)
    W1 = np.asarray(W1, np.float32)
    b1 = np.asarray(b1, np.float32)

    w1tb1 = np.concatenate([W1.T, b1[None, :]], 0).astype(np.float32)
    ident = np.eye(64, dtype=np.float32)
    maskx = np.zeros((128, KNN, 16, 3), np.float32)
    for p in range(128):
        maskx[p, :, p % 16, :] = 1.0
    maskx = maskx.reshape(128, KNN * 16 * 3)

    in_maps = []
    for b in range(B):
        xb = np.ascontiguousarray(xyz[b])
        fb = np.concatenate([features[b], np.ones((1, N), np.float32)], 0)
        in_maps.append({
            "xyzb": np.ascontiguousarray(np.tile(xb[None], (128, 1, 1))),
            "xyzp": np.ascontiguousarray(np.tile(xb.T[None], (128, 1, 1))),
            "qbuf": np.ascontiguousarray(
                xyz_fp[b].reshape(NT, 128, 3).transpose(1, 0, 2)),
            "maskx": maskx,
            "feat65": np.ascontiguousarray(fb),
            "w1tb1": w1tb1,
            "ident": ident,
        })
    return in_maps


def run(xyz, xyz_fp, features, W1, b1, trace=False):
    nc = _get_nc()
    in_maps = _prep_in_maps(xyz, xyz_fp, features, W1, b1)
    res = run_bass_kernel_spmd(nc, in_maps, core_ids=list(range(B)), trace=trace)
    feats = np.empty((B, N, DM), np.float32)
    knn = np.empty((B, M, KNN, 3), np.float32)
    x = np.empty((B, N, DM), np.float32)
    for b in range(B):
        r = res.results[b]
        feats[b] = r["feats_out"].transpose(1, 0, 2).reshape(N, DM)
        x[b] = r["x_out"].transpose(1, 0, 2).reshape(N, DM)
        knn[b] = r["knn_out"].transpose(1, 0, 2, 3).reshape(M, KNN, 3)
    return (feats, knn, x), res


def kernel(xyz, xyz_fp, features, features_fp=None, W1=None, b1=None, k=16,
           **_ignored):
    assert int(k) == KNN
    (feats, knn, x), _ = run(xyz, xyz_fp, features, W1, b1, trace=False)
    return feats, knn, x


# revision 13
# speedup vs baseline: 1.3412x; 1.1924x over previous
"""Trainium2 Bass kernel for nn_Adaptive_FP (retrieval KNN), 8 NeuronCores.

Sharding: data-parallel over B (8 batches -> 8 cores). Per core:
  - feats = features.T              via PE transposes
  - x = feats @ W1.T + b1           via PE matmul (ones-row bias trick)
  - knn: EXACT fp32 d2 for all 2048 candidates per query tile:
      ACT computes (q_c - p_c)^2 per coordinate (same fp32 rounding as the
      reference), DMA CCE-add accumulates the three squares, DVE negates
      (2x mode) and runs the flat top-16 via max8/max_index/match_replace
      (positions are global indices, already rank-ordered), gpsimd ap_gather
      fetches winner xyz rows (16-partition-group shared lists), and a
      mask-multiply + reduce-add extracts each query's own rows.
knn matches stable argsort of the reference's fp32 d2 bit-exactly (up to
fp32-tie degeneracies).
"""
import sys
import numpy as np

sys.path.insert(0, "/opt/trn_rl_repo")

def _install_ntff_hook_shim():
    """Provide antenv.axon_hooks (absent from this image's antenv stub) so
    run_bass_kernel_spmd(trace=True) can fetch the NTFF profiling hook."""
    import types
    if "antenv.axon_hooks" in sys.modules:
        return
    mod = types.ModuleType("antenv.axon_hooks")
    state = {"hook": None, "tried": False}

    def set_axon_ntff_profile_hook(hook):
        state["hook"] = hook

    def get_axon_ntff_profile_hook():
        if state["hook"] is None and not state["tried"]:
            state["tried"] = True
            try:
                sys.path.insert(0, "/root/.axon_site/trn_agent_boot")
                from trn_boot import _ntff_profile_via_ctypes
                state["hook"] = _ntff_profile_via_ctypes(
                    "/opt/axon/libaxon_pjrt.so")
            except Exception:
                state["hook"] = None
        return state["hook"]

    mod.set_axon_ntff_profile_hook = set_axon_ntff_profile_hook
    mod.get_axon_ntff_profile_hook = get_axon_ntff_profile_hook
    sys.modules["antenv.axon_hooks"] = mod
    try:
        import antenv
        antenv.axon_hooks = mod
    except Exception:
        pass


_install_ntff_hook_shim()

import concourse.bass as bass
import concourse.bacc as bacc
import concourse.mybir as mybir
from concourse.tile import TileContext
from concourse.bass_utils import run_bass_kernel_spmd

F32 = mybir.dt.float32
U16 = mybir.dt.uint16
I16 = mybir.dt.int16
ALU = mybir.AluOpType
AXX = mybir.AxisListType.X
SQUARE = mybir.ActivationFunctionType.Square
CPY = mybir.ActivationFunctionType.Copy

B, N, M, CP, DM, KNN = 8, 2048, 8192, 64, 64, 16
NT = M // 128            # 64 query tiles per core
NEG = -3.0e38            # finite -inf sentinel for match_replace


def build_nc():
    nc = bacc.Bacc()

    # ---- external inputs ----
    xyzb_d = nc.declare_dram_parameter("xyzb", [128, N, 3], F32, isOutput=False)
    xyzp_d = nc.declare_dram_parameter("xyzp", [128, 3, N], F32, isOutput=False)
    qbuf_d = nc.declare_dram_parameter("qbuf", [128, NT, 3], F32, isOutput=False)
    mask_d = nc.declare_dram_parameter("maskx", [128, KNN * 16 * 3], F32,
                                       isOutput=False)
    feat_d = nc.declare_dram_parameter("feat65", [CP + 1, N], F32, isOutput=False)
    w1_d = nc.declare_dram_parameter("w1tb1", [CP + 1, DM], F32, isOutput=False)
    id_d = nc.declare_dram_parameter("ident", [64, 64], F32, isOutput=False)

    # ---- external outputs ----
    knn_o = nc.declare_dram_parameter("knn_out", [128, NT, KNN, 3], F32,
                                      isOutput=True)
    feats_o = nc.declare_dram_parameter("feats_out", [128, N // 128, DM], F32,
                                        isOutput=True)
    x_o = nc.declare_dram_parameter("x_out", [128, N // 128, DM], F32,
                                    isOutput=True)

    with TileContext(nc) as tc:
        with tc.tile_pool(name="cn", bufs=1) as cn, \
             tc.tile_pool(name="io", bufs=1) as io, \
             tc.tile_pool(name="work", bufs=2) as wk, \
             tc.tile_pool(name="wsm", bufs=3) as wsm, \
             tc.tile_pool(name="psS", bufs=2, space="PSUM") as psS:

            # ---------------- load inputs ----------------
            xyzb = cn.tile([128, N, 3], F32)
            xyzp = cn.tile([128, 3, N], F32)
            qbuf = cn.tile([128, NT, 3], F32)
            maskx = cn.tile([128, KNN * 16 * 3], F32)
            feat_s = cn.tile([CP + 1, N], F32)
            w1_s = cn.tile([CP + 1, DM], F32)
            id_s = cn.tile([64, 64], F32)
            for dst, src in [(xyzb, xyzb_d), (xyzp, xyzp_d),
                             (qbuf, qbuf_d), (maskx, mask_d),
                             (feat_s, feat_d), (w1_s, w1_d), (id_s, id_d)]:
                nc.sync.dma_start(out=dst[:], in_=src[:])

            # ---------------- feats / x (PE) ----------------
            featsb = io.tile([128, N // 128, DM], F32)
            xb = io.tile([128, N // 128, DM], F32)
            for kk in range(N // 128):
                psf = psS.tile([128, DM], F32, tag="psf")
                nc.tensor.transpose(psf[:], feat_s[0:CP, kk * 128:(kk + 1) * 128],
                                    id_s[:])
                nc.scalar.activation(out=featsb[:, kk], in_=psf[:], func=CPY)
                psx = psS.tile([128, DM], F32, tag="psx")
                nc.tensor.matmul(psx[:], feat_s[:, kk * 128:(kk + 1) * 128],
                                 w1_s[:], start=True, stop=True)
                nc.scalar.activation(out=xb[:, kk], in_=psx[:], func=CPY)
            nc.sync.dma_start(out=feats_o[:], in_=featsb[:])
            nc.sync.dma_start(out=x_o[:], in_=xb[:])

            # ---------------- main loop ----------------
            knn_buf = io.tile([128, NT, KNN, 3], F32)
            for t in range(NT):
                # exact squares per coordinate; acc accumulates via DMA CCE add
                acc = wk.tile([128, N], F32, tag="acc")
                sq1 = wk.tile([128, N], F32, tag="sq1")
                sq2 = wk.tile([128, N], F32, tag="sq2")
                for c, out_t in ((0, acc), (1, sq1), (2, sq2)):
                    nc.scalar.activation(
                        out=out_t[:], in_=xyzp[:, c], func=SQUARE,
                        bias=qbuf[:, t, c:c + 1], scale=-1.0)
                # acc = (sq0 + sq1) via DMA CCE add (reference rounding order)
                nc.gpsimd.dma_start(out=acc[:], in_=sq1[:], accum_op=ALU.add)
                # negd2 = (-acc) - sq2 == -((sq0+sq1)+sq2), exact
                negd2 = wk.tile([128, N], F32, tag="negd2")
                nc.vector.scalar_tensor_tensor(
                    out=negd2[:], in0=acc[:], scalar=-1.0, in1=sq2[:],
                    op0=ALU.mult, op1=ALU.subtract)

                # flat exact top-16: positions are global ids, rank-ordered
                v8 = wsm.tile([128, KNN], F32, tag="v8")
                pos16 = wsm.tile([128, KNN], U16, tag="pos16")
                nc.vector.max(v8[:, 0:8], negd2[:])
                nc.vector.max_index(pos16[:, 0:8], v8[:, 0:8], negd2[:])
                nc.vector.match_replace(negd2[:], v8[:, 0:8], negd2[:], NEG)
                nc.vector.max(v8[:, 8:16], negd2[:])
                nc.vector.max_index(pos16[:, 8:16], v8[:, 8:16], negd2[:])

                # gather winner coords (16-partition-group shared lists)
                gath = wsm.tile([128, KNN * 16, 3], F32, tag="gath")
                nc.gpsimd.ap_gather(out_ap=gath[:], in_ap=xyzb[:],
                                    idxs_ap=pos16.bitcast(I16)[:],
                                    channels=128, num_elems=N, d=3,
                                    num_idxs=KNN * 16)
                # extract own query's rows: multiply by one-hot mask, reduce q'
                tmp = wsm.tile([128, KNN * 16, 3], F32, tag="tmp")
                nc.vector.scalar_tensor_tensor(
                    out=tmp.rearrange("p i c -> p (i c)"),
                    in0=gath.rearrange("p i c -> p (i c)"), scalar=0.0,
                    in1=maskx[:], op0=ALU.add, op1=ALU.mult)
                nc.vector.tensor_reduce(
                    out=knn_buf[:, t],
                    in_=tmp.rearrange("p (j q) c -> p j c q", q=16),
                    op=ALU.add, axis=AXX)

            nc.sync.dma_start(out=knn_o[:], in_=knn_buf[:])

    nc.compile()
    return nc


_NC_CACHE = {}


def _get_nc():
    if "nc" not in _NC_CACHE:
        _NC_CACHE["nc"] = build_nc()
    return _NC_CACHE["nc"]


def _prep_in_maps(xyz, xyz_fp, features, W1, b1):
    xyz = np.asarray(xyz, np.float32)
    xyz_fp = np.asarray(xyz_fp, np.float32)
    features = np.asarray(features, np.float32)
    W1 = np.asarray(W1, np.float32)
    b1 = np.asarray(b1, np.float32)

    w1tb1 = np.concatenate([W1.T, b1[None, :]], 0).astype(np.float32)
    ident = np.eye(64, dtype=np.float32)
    maskx = np.zeros((128, KNN, 16, 3), np.float32)
    for p in range(128):
        maskx[p, :, p % 16, :] = 1.0
    maskx = maskx.reshape(128, KNN * 16 * 3)

    in_maps = []
    for b in range(B):
        xb = np.ascontiguousarray(xyz[b])
        fb = np.concatenate([features[b], np.ones((1, N), np.float32)], 0)
        in_maps.append({
            "xyzb": np.ascontiguousarray(np.tile(xb[None], (128, 1, 1))),
            "xyzp": np.ascontiguousarray(np.tile(xb.T[None], (128, 1, 1))),
            "qbuf": np.ascontiguousarray(
                xyz_fp[b].reshape(NT, 128, 3).transpose(1, 0, 2)),
            "maskx": maskx,
            "feat65": np.ascontiguousarray(fb),
            "w1tb1": w1tb1,
            "ident": ident,
        })
    return in_maps


def run(xyz, xyz_fp, features, W1, b1, trace=False):
    nc = _get_nc()
    in_maps = _prep_in_maps(xyz, xyz_fp, features, W1, b1)
    res = run_bass_kernel_spmd(nc, in_maps, core_ids=list(range(B)), trace=trace)
    feats = np.empty((B, N, DM), np.float32)
    knn = np.empty((B, M, KNN, 3), np.float32)
    x = np.empty((B, N, DM), np.float32)
    for b in range(B):
        r = res.results[b]
        feats[b] = r["feats_out"].transpose(1, 0, 2).reshape(N, DM)
        x[b] = r["x_out"].transpose(1, 0, 2).reshape(N, DM)
        knn[b] = r["knn_out"].transpose(1, 0, 2, 3).reshape(M, KNN, 3)
    return (feats, knn, x), res


def kernel(xyz, xyz_fp, features, features_fp=None, W1=None, b1=None, k=16,
           **_ignored):
    assert int(k) == KNN
    (feats, knn, x), _ = run(xyz, xyz_fp, features, W1, b1, trace=False)
    return feats, knn, x


# revision 16
# speedup vs baseline: 1.3473x; 1.0046x over previous
"""Trainium2 Bass kernel for nn_Adaptive_FP (retrieval KNN), 8 NeuronCores.

Sharding: data-parallel over B (8 batches -> 8 cores). Per core:
  - feats = features.T              via PE transposes
  - x = feats @ W1.T + b1           via PE matmul (ones-row bias trick)
  - knn: EXACT fp32 d2 for all 2048 candidates per query tile:
      ACT computes (q_c - p_c)^2 per coordinate (same fp32 rounding as the
      reference), DMA CCE-add accumulates the three squares, DVE negates
      (2x mode) and runs the flat top-16 via max8/max_index/match_replace
      (positions are global indices, already rank-ordered), gpsimd ap_gather
      fetches winner xyz rows (16-partition-group shared lists), and a
      mask-multiply + reduce-add extracts each query's own rows.
knn matches stable argsort of the reference's fp32 d2 bit-exactly (up to
fp32-tie degeneracies).
"""
import sys
import numpy as np

sys.path.insert(0, "/opt/trn_rl_repo")

def _install_ntff_hook_shim():
    """Provide antenv.axon_hooks (absent from this image's antenv stub) so
    run_bass_kernel_spmd(trace=True) can fetch the NTFF profiling hook."""
    import types
    if "antenv.axon_hooks" in sys.modules:
        return
    mod = types.ModuleType("antenv.axon_hooks")
    state = {"hook": None, "tried": False}

    def set_axon_ntff_profile_hook(hook):
        state["hook"] = hook

    def get_axon_ntff_profile_hook():
        if state["hook"] is None and not state["tried"]:
            state["tried"] = True
            try:
                sys.path.insert(0, "/root/.axon_site/trn_agent_boot")
                from trn_boot import _ntff_profile_via_ctypes
                state["hook"] = _ntff_profile_via_ctypes(
                    "/opt/axon/libaxon_pjrt.so")
            except Exception:
                state["hook"] = None
        return state["hook"]

    mod.set_axon_ntff_profile_hook = set_axon_ntff_profile_hook
    mod.get_axon_ntff_profile_hook = get_axon_ntff_profile_hook
    sys.modules["antenv.axon_hooks"] = mod
    try:
        import antenv
        antenv.axon_hooks = mod
    except Exception:
        pass


_install_ntff_hook_shim()

import concourse.bass as bass
import concourse.bacc as bacc
import concourse.mybir as mybir
from concourse.tile import TileContext
from concourse.bass_utils import run_bass_kernel_spmd

F32 = mybir.dt.float32
U16 = mybir.dt.uint16
I16 = mybir.dt.int16
ALU = mybir.AluOpType
AXX = mybir.AxisListType.X
SQUARE = mybir.ActivationFunctionType.Square
CPY = mybir.ActivationFunctionType.Copy

B, N, M, CP, DM, KNN = 8, 2048, 8192, 64, 64, 16
NT = M // 128            # 64 query tiles per core
NEG = -3.0e38            # finite -inf sentinel for match_replace


def build_nc():
    nc = bacc.Bacc()

    # ---- external inputs ----
    xyzb_d = nc.declare_dram_parameter("xyzb", [128, N, 3], F32, isOutput=False)
    xyzp_d = nc.declare_dram_parameter("xyzp", [128, 3, N], F32, isOutput=False)
    qbuf_d = nc.declare_dram_parameter("qbuf", [128, NT, 3], F32, isOutput=False)
    mask_d = nc.declare_dram_parameter("maskx", [128, KNN * 16 * 3], F32,
                                       isOutput=False)
    feat_d = nc.declare_dram_parameter("feat65", [CP + 1, N], F32, isOutput=False)
    w1_d = nc.declare_dram_parameter("w1tb1", [CP + 1, DM], F32, isOutput=False)
    id_d = nc.declare_dram_parameter("ident", [64, 64], F32, isOutput=False)

    # ---- external outputs ----
    knn_o = nc.declare_dram_parameter("knn_out", [128, NT, KNN, 3], F32,
                                      isOutput=True)
    feats_o = nc.declare_dram_parameter("feats_out", [128, N // 128, DM], F32,
                                        isOutput=True)
    x_o = nc.declare_dram_parameter("x_out", [128, N // 128, DM], F32,
                                    isOutput=True)

    with TileContext(nc) as tc:
        with tc.tile_pool(name="cn", bufs=1) as cn, \
             tc.tile_pool(name="io", bufs=1) as io, \
             tc.tile_pool(name="work", bufs=2) as wk, \
             tc.tile_pool(name="wsm", bufs=3) as wsm, \
             tc.tile_pool(name="psS", bufs=2, space="PSUM") as psS:

            # ---------------- load inputs ----------------
            xyzb = cn.tile([128, N, 3], F32)
            xyzp = cn.tile([128, 3, N], F32)
            qbuf = cn.tile([128, NT, 3], F32)
            maskx = cn.tile([128, KNN * 16 * 3], F32)
            feat_s = cn.tile([CP + 1, N], F32)
            w1_s = cn.tile([CP + 1, DM], F32)
            id_s = cn.tile([64, 64], F32)
            for dst, src in [(xyzb, xyzb_d), (xyzp, xyzp_d),
                             (qbuf, qbuf_d), (maskx, mask_d),
                             (feat_s, feat_d), (w1_s, w1_d), (id_s, id_d)]:
                nc.sync.dma_start(out=dst[:], in_=src[:])

            # ---------------- feats / x (PE) ----------------
            featsb = io.tile([128, N // 128, DM], F32)
            xb = io.tile([128, N // 128, DM], F32)
            for kk in range(N // 128):
                psf = psS.tile([128, DM], F32, tag="psf")
                nc.tensor.transpose(psf[:], feat_s[0:CP, kk * 128:(kk + 1) * 128],
                                    id_s[:])
                nc.scalar.activation(out=featsb[:, kk], in_=psf[:], func=CPY)
                psx = psS.tile([128, DM], F32, tag="psx")
                nc.tensor.matmul(psx[:], feat_s[:, kk * 128:(kk + 1) * 128],
                                 w1_s[:], start=True, stop=True)
                nc.scalar.activation(out=xb[:, kk], in_=psx[:], func=CPY)
            nc.sync.dma_start(out=feats_o[:], in_=featsb[:])
            nc.sync.dma_start(out=x_o[:], in_=xb[:])

            # ---------------- main loop (extract skewed 1 tile) ----------------
            knn_buf = io.tile([128, NT, KNN, 3], F32)
            gaths = {}
            for t in range(NT + 1):
                if t < NT:
                    # exact squares per coordinate
                    acc = wk.tile([128, N], F32, tag="acc")
                    sq1 = wk.tile([128, N], F32, tag="sq1")
                    sq2 = wk.tile([128, N], F32, tag="sq2")
                    for c, out_t in ((0, acc), (1, sq1), (2, sq2)):
                        nc.scalar.activation(
                            out=out_t[:], in_=xyzp[:, c], func=SQUARE,
                            bias=qbuf[:, t, c:c + 1], scale=-1.0)
                    # acc = (sq0 + sq1) via DMA CCE add (reference rounding)
                    nc.gpsimd.dma_start(out=acc[:], in_=sq1[:], accum_op=ALU.add)
                    # negd2 = (-acc) - sq2 == -((sq0+sq1)+sq2), exact
                    negd2 = wk.tile([128, N], F32, tag="negd2")
                    nc.vector.scalar_tensor_tensor(
                        out=negd2[:], in0=acc[:], scalar=-1.0, in1=sq2[:],
                        op0=ALU.mult, op1=ALU.subtract)

                    # flat exact top-16: positions are global, rank-ordered
                    v8 = wsm.tile([128, KNN], F32, tag="v8")
                    pos16 = wsm.tile([128, KNN], U16, tag="pos16")
                    nc.vector.max(v8[:, 0:8], negd2[:])
                    nc.vector.max_index(pos16[:, 0:8], v8[:, 0:8], negd2[:])
                    nc.vector.match_replace(negd2[:], v8[:, 0:8], negd2[:], NEG)
                    nc.vector.max(v8[:, 8:16], negd2[:])
                    nc.vector.max_index(pos16[:, 8:16], v8[:, 8:16], negd2[:])

                    # gather winner coords (16-partition-group shared lists)
                    gath = wsm.tile([128, KNN * 16, 3], F32, tag="gath")
                    nc.gpsimd.ap_gather(out_ap=gath[:], in_ap=xyzb[:],
                                        idxs_ap=pos16.bitcast(I16)[:],
                                        channels=128, num_elems=N, d=3,
                                        num_idxs=KNN * 16)
                    gaths[t] = gath
                if t >= 1:
                    # extract previous tile's rows: one-hot mask, reduce q'
                    gp = gaths.pop(t - 1)
                    tmp = wsm.tile([128, KNN * 16, 3], F32, tag="tmp")
                    nc.vector.scalar_tensor_tensor(
                        out=tmp.rearrange("p i c -> p (i c)"),
                        in0=gp.rearrange("p i c -> p (i c)"), scalar=0.0,
                        in1=maskx[:], op0=ALU.add, op1=ALU.mult)
                    nc.vector.tensor_reduce(
                        out=knn_buf[:, t - 1],
                        in_=tmp.rearrange("p (j q) c -> p j c q", q=16),
                        op=ALU.add, axis=AXX)

            nc.sync.dma_start(out=knn_o[:], in_=knn_buf[:])

    nc.compile()
    return nc


_NC_CACHE = {}


def _get_nc():
    if "nc" not in _NC_CACHE:
        _NC_CACHE["nc"] = build_nc()
    return _NC_CACHE["nc"]


def _prep_in_maps(xyz, xyz_fp, features, W1, b1):
    xyz = np.asarray(xyz, np.float32)
    xyz_fp = np.asarray(xyz_fp, np.float32)
    features = np.asarray(features, np.float32)
    W1 = np.asarray(W1, np.float32)
    b1 = np.asarray(b1, np.float32)

    w1tb1 = np.concatenate([W1.T, b1[None, :]], 0).astype(np.float32)
    ident = np.eye(64, dtype=np.float32)
    maskx = np.zeros((128, KNN, 16, 3), np.float32)
    for p in range(128):
        maskx[p, :, p % 16, :] = 1.0
    maskx = maskx.reshape(128, KNN * 16 * 3)

    in_maps = []
    for b in range(B):
        xb = np.ascontiguousarray(xyz[b])
        fb = np.concatenate([features[b], np.ones((1, N), np.float32)], 0)
        in_maps.append({
            "xyzb": np.ascontiguousarray(np.tile(xb[None], (128, 1, 1))),
            "xyzp": np.ascontiguousarray(np.tile(xb.T[None], (128, 1, 1))),
            "qbuf": np.ascontiguousarray(
                xyz_fp[b].reshape(NT, 128, 3).transpose(1, 0, 2)),
            "maskx": maskx,
            "feat65": np.ascontiguousarray(fb),
            "w1tb1": w1tb1,
            "ident": ident,
        })
    return in_maps


def run(xyz, xyz_fp, features, W1, b1, trace=False):
    nc = _get_nc()
    in_maps = _prep_in_maps(xyz, xyz_fp, features, W1, b1)
    res = run_bass_kernel_spmd(nc, in_maps, core_ids=list(range(B)), trace=trace)
    feats = np.empty((B, N, DM), np.float32)
    knn = np.empty((B, M, KNN, 3), np.float32)
    x = np.empty((B, N, DM), np.float32)
    for b in range(B):
        r = res.results[b]
        feats[b] = r["feats_out"].transpose(1, 0, 2).reshape(N, DM)
        x[b] = r["x_out"].transpose(1, 0, 2).reshape(N, DM)
        knn[b] = r["knn_out"].transpose(1, 0, 2, 3).reshape(M, KNN, 3)
    return (feats, knn, x), res


def kernel(xyz, xyz_fp, features, features_fp=None, W1=None, b1=None, k=16,
           **_ignored):
    assert int(k) == KNN
    (feats, knn, x), _ = run(xyz, xyz_fp, features, W1, b1, trace=False)
    return feats, knn, x
